# revision 8
# baseline (speedup 1.0000x reference)
"""Multi-head attention (B=2, S=2048, D=1024, H=16, Dh=64) on 8 trn2 cores.

Sharding: core c handles batch b = c//4 and head-group g = c%4 (4 heads).
Each core:
  - projects q/k (transposed layout [dh, S]) and v (natural [S, dh]) with
    fp32r matmuls,
  - computes simT = k^T q (keys on partitions) row-packed 2 heads per
    PE pass,
  - exp on ScalarE (scale=1/sqrt(dh) folded in; no max-subtraction: scores
    are ~N(0,1) so exp cannot overflow),
  - mask multiply on DVE in bf16,
  - PV matmul in bf16 with an appended ones-column (M=65) whose output row
    is the softmax denominator,
  - normalizes via reciprocal + PE ones-broadcast, then projects with Wo.
Host sums the 4 head-group partials per batch and adds bo.
"""

import os
import sys

for _p in ("/opt/trn_rl_repo", "/root/.axon_site/_ro/trn_rl_repo"):
    if os.path.isdir(_p) and _p not in sys.path:
        sys.path.append(_p)

from contextlib import ExitStack

import ml_dtypes
import numpy as np

import concourse.bass as bass
import concourse.tile as tile
from concourse import bacc
from concourse import mybir

F32 = mybir.dt.float32
F32R = mybir.dt.float32r
BF16 = mybir.dt.bfloat16
AF = mybir.ActivationFunctionType


def build_attention_nc(S=2048, D=1024, HL=4, DH=64, debug=False):
    """Bass program for one core: 4 local heads of one batch.

    Inputs : xT [D, S] f32, maskT [S, S] bf16, Wq/Wk/Wv [D, HL*DH] f32,
             Wo [HL*DH, D] f32
    Output : out [S, D] f32 (partial: this head-group's contribution, no bias)
    """
    QB = min(512, S)  # q-chunk width (moving free dim)
    KB = 128          # key tile (partition dim)
    INNER = HL * DH   # local inner dim (256)
    NP = D // 128     # contraction tiles over D
    NQ = S // QB      # q chunks
    NK = S // KB      # key tiles
    NQT = S // 128    # out-proj q tiles
    NH = max(D // 512, 1)  # out-proj N chunks
    NB = min(512, D)  # out-proj N width
    NHP = HL // 2     # head pairs
    scale = float(DH) ** -0.5

    assert HL % 2 == 0 and DH == 64 and D % 128 == 0 and S % 512 == 0

    nc = bacc.Bacc(trn_type="TRN2")

    xT_d = nc.dram_tensor("xT", (D, S), F32R, kind="ExternalInput")
    maskT_d = nc.dram_tensor("maskT", (S, S), BF16, kind="ExternalInput")
    wq_d = nc.dram_tensor("Wq", (D, INNER), F32R, kind="ExternalInput")
    wk_d = nc.dram_tensor("Wk", (D, INNER), F32R, kind="ExternalInput")
    wv_d = nc.dram_tensor("Wv", (D, INNER), F32R, kind="ExternalInput")
    wo_d = nc.dram_tensor("Wo", (INNER, D), F32R, kind="ExternalInput")
    out_d = nc.dram_tensor("out", (S, D), F32, kind="ExternalOutput")
    if debug:
        dbg_qT = nc.dram_tensor("dbg_qT", (128, NHP * S), F32R, kind="ExternalOutput")
        dbg_kT = nc.dram_tensor("dbg_kT", (128, NHP * S), F32R, kind="ExternalOutput")
        dbg_v = nc.dram_tensor("dbg_v", (128, NK * HL * (DH + 1)), BF16, kind="ExternalOutput")
        dbg_sim = nc.dram_tensor("dbg_sim", (128, 2 * QB), F32, kind="ExternalOutput")
        dbg_e = nc.dram_tensor("dbg_e", (128, 2 * QB), BF16, kind="ExternalOutput")
        dbg_pv = nc.dram_tensor("dbg_pv", (DH + 1, QB), F32, kind="ExternalOutput")
        dbg_rc = nc.dram_tensor("dbg_rc", (1, QB), F32, kind="ExternalOutput")
        dbg_bc = nc.dram_tensor("dbg_bc", (64, QB), F32, kind="ExternalOutput")
        dbg_on = nc.dram_tensor("dbg_on", (128, NHP * S), F32R, kind="ExternalOutput")

    with tile.TileContext(nc) as tc, ExitStack() as ctx:
        persist = ctx.enter_context(tc.tile_pool(name="persist", bufs=1))

        # persistent SBUF tensors
        qT = persist.tile([128, NHP, S], F32R)   # [2x64 dh, hp, q]
        kT = persist.tile([128, NHP, S], F32R)
        v_sb = persist.tile([128, NK, HL, DH + 1], BF16)  # v + ones col
        wo_sb = persist.tile([128, NHP, D], F32R)
        o_norm = persist.tile([128, NHP, S], F32R)  # normalized attn out ^T
        ones_sb = persist.tile([1, 64], F32)

        nc.vector.memset(ones_sb[:, :], 1.0)
        # ones columns of v_aug (overwritten except col DH by the v copies)
        nc.vector.memset(v_sb[:, :, :, :], 1.0)

        for n in range(NHP):
            nc.sync.dma_start(
                out=wo_sb[:, n, :], in_=wo_d[n * 128 : (n + 1) * 128, :]
            )

        # ---------------- phase 1: projections ----------------
        with (
            tc.tile_pool(name="ph1", bufs=1) as ph1,
            tc.tile_pool(name="p1ps", bufs=6, space="PSUM") as p1ps,
        ):
            xts = ph1.tile([128, NP, S], F32R)
            wq_sb = ph1.tile([128, NP, INNER], F32R)
            wk_sb = ph1.tile([128, NP, INNER], F32R)
            wv_sb = ph1.tile([128, NP, INNER], F32R)
            for p in range(NP):
                nc.sync.dma_start(out=xts[:, p, :], in_=xT_d[p * 128 : (p + 1) * 128, :])
                nc.sync.dma_start(out=wq_sb[:, p, :], in_=wq_d[p * 128 : (p + 1) * 128, :])
                nc.sync.dma_start(out=wk_sb[:, p, :], in_=wk_d[p * 128 : (p + 1) * 128, :])
                nc.sync.dma_start(out=wv_sb[:, p, :], in_=wv_d[p * 128 : (p + 1) * 128, :])

            # q/k projections, transposed: psum[dh-block, q] = W_chunk^T @ xT
            for w_sb, dst, eng in ((wq_sb, qT, nc.scalar), (wk_sb, kT, nc.scalar)):
                for hp in range(NHP):
                    ps_l = [
                        p1ps.tile([128, QB], F32, tag="p1", name=f"ps_{hp}_{i}")
                        for i in range(NQ)
                    ]
                    for p in range(NP):
                        for qt in range(NQ):
                            nc.tensor.matmul(
                                ps_l[qt][:, :],
                                lhsT=w_sb[:, p, hp * 128 : (hp + 1) * 128],
                                rhs=xts[:, p, qt * QB : (qt + 1) * QB],
                                start=(p == 0),
                                stop=(p == NP - 1),
                            )
                    for qt in range(NQ):
                        eng.copy(dst[:, hp, qt * QB : (qt + 1) * QB], ps_l[qt][:, :])

            # v projection, natural: psum[key-block, inner] = xT_chunk^T @ Wv
            for kt in range(NK):
                ps_v = p1ps.tile([128, INNER], F32, tag="p1")
                for p in range(NP):
                    nc.tensor.matmul(
                        ps_v[:, :],
                        lhsT=xts[:, p, kt * 128 : (kt + 1) * 128],
                        rhs=wv_sb[:, p, :],
                        start=(p == 0),
                        stop=(p == NP - 1),
                    )
                nc.vector.tensor_copy(
                    v_sb[:, kt, :, 0:DH],
                    ps_v[:, :].rearrange("p (h d) -> p h d", h=HL),
                )

        if debug:
            nc.sync.dma_start(out=dbg_qT[:, :], in_=qT[:, :, :])
            nc.sync.dma_start(out=dbg_kT[:, :], in_=kT[:, :, :])
            nc.sync.dma_start(out=dbg_v[:, :], in_=v_sb[:, :, :, :])

        # ---------------- phase 2: attention ----------------
        with (
            tc.tile_pool(name="mpool", bufs=4) as mpool,
            tc.tile_pool(name="epool", bufs=4) as epool,
            tc.tile_pool(name="npool", bufs=4) as npool,
            tc.tile_pool(name="opool", bufs=2) as opool,
            tc.tile_pool(name="simps", bufs=2, space="PSUM") as simps,
            tc.tile_pool(name="pvps", bufs=2, space="PSUM") as pvps,
            tc.tile_pool(name="bcps", bufs=1, space="PSUM") as bcps,
            tc.tile_pool(name="prjps", bufs=1, space="PSUM") as prjps,
        ):
            for hp in range(NHP):
                for qt in range(NQ):
                    pv_e = pvps.tile([DH + 1, QB], F32, tag="pv")
                    pv_o = pvps.tile([DH + 1, QB], F32, tag="pv")
                    for kt in range(NK):
                        m_t = mpool.tile([128, QB], BF16, tag="m")
                        nc.sync.dma_start(
                            out=m_t[:, :],
                            in_=maskT_d[kt * 128 : (kt + 1) * 128, qt * QB : (qt + 1) * QB],
                        )
                        ps = simps.tile([128, 2 * QB], F32, tag="sim")
                        nc.tensor.matmul(
                            ps[:, 0:QB],
                            lhsT=kT[0:64, hp, kt * 128 : (kt + 1) * 128],
                            rhs=qT[0:64, hp, qt * QB : (qt + 1) * QB],
                            start=True,
                            stop=True,
                            tile_position=(0, 0),
                        )
                        nc.tensor.matmul(
                            ps[:, QB : 2 * QB],
                            lhsT=kT[64:128, hp, kt * 128 : (kt + 1) * 128],
                            rhs=qT[64:128, hp, qt * QB : (qt + 1) * QB],
                            start=True,
                            stop=True,
                            tile_position=(64, 0),
                        )
                        e_t = epool.tile([128, 2 * QB], BF16, tag="e")
                        if debug and hp == 0 and qt == 0 and kt == 0:
                            s_sb = epool.tile([128, 2 * QB], F32, name="s_sb", bufs=1)
                            nc.vector.tensor_copy(s_sb[:, :], ps[:, :])
                            nc.sync.dma_start(out=dbg_sim[:, :], in_=s_sb[:, :])
                        nc.scalar.activation(e_t[:, :], ps[:, :], AF.Exp, scale=scale)
                        nc.vector.tensor_mul(e_t[:, 0:QB], e_t[:, 0:QB], m_t[:, :])
                        nc.vector.tensor_mul(e_t[:, QB : 2 * QB], e_t[:, QB : 2 * QB], m_t[:, :])
                        nc.tensor.matmul(
                            pv_e[:, :],
                            lhsT=v_sb[:, kt, 2 * hp + 0, :],
                            rhs=e_t[:, 0:QB],
                            start=(kt == 0),
                            stop=(kt == NK - 1),
                        )
                        nc.tensor.matmul(
                            pv_o[:, :],
                            lhsT=v_sb[:, kt, 2 * hp + 1, :],
                            rhs=e_t[:, QB : 2 * QB],
                            start=(kt == 0),
                            stop=(kt == NK - 1),
                        )
                    if debug and hp == 0 and qt == 0:
                        nc.sync.dma_start(out=dbg_e[:, :], in_=e_t[:, :])
                        pv_sb = epool.tile([DH + 1, QB], F32, name="pv_sb", bufs=1)
                        nc.vector.tensor_copy(pv_sb[:, :], pv_e[:, :])
                        nc.sync.dma_start(out=dbg_pv[:, :], in_=pv_sb[:, :])
                    for hl, pv in ((0, pv_e), (1, pv_o)):
                        dn_t = npool.tile([1, QB], F32, tag="dn")
                        nc.scalar.copy(dn_t[:, :], pv[DH : DH + 1, :])
                        rc_t = npool.tile([1, QB], F32, tag="rc")
                        nc.vector.reciprocal_approx_fast(out=rc_t[:, :], in_=dn_t[:, :])
                        bc_ps = bcps.tile([64, QB], F32, tag="bc")
                        nc.tensor.matmul(
                            bc_ps[:, :],
                            lhsT=ones_sb[:, :],
                            rhs=rc_t[:, :],
                            start=True,
                            stop=True,
                        )
                        bc_sb = npool.tile([64, QB], F32, tag="bcs")
                        nc.scalar.copy(bc_sb[:, :], bc_ps[:, :])
                        if debug and hp == 0 and qt == 0 and hl == 0:
                            nc.sync.dma_start(out=dbg_rc[:, :], in_=rc_t[:, :])
                            nc.sync.dma_start(out=dbg_bc[:, :], in_=bc_sb[:, :])
                        nc.vector.tensor_mul(
                            o_norm[hl * 64 : (hl + 1) * 64, hp, qt * QB : (qt + 1) * QB],
                            pv[0:DH, :],
                            bc_sb[:, :],
                        )

            if debug:
                nc.sync.dma_start(out=dbg_on[:, :], in_=o_norm[:, :, :])

            # ---------------- output projection ----------------
            for qt in range(NQT):
                o_sb = opool.tile([128, D], F32, tag="o")
                for nh in range(NH):
                    fo = prjps.tile([128, NB], F32, tag="prj")
                    for hp in range(NHP):
                        nc.tensor.matmul(
                            fo[:, :],
                            lhsT=o_norm[:, hp, qt * 128 : (qt + 1) * 128],
                            rhs=wo_sb[:, hp, nh * NB : (nh + 1) * NB],
                            start=(hp == 0),
                            stop=(hp == NHP - 1),
                        )
                    if nh % 2 == 0:
                        nc.scalar.copy(o_sb[:, nh * NB : (nh + 1) * NB], fo[:, :])
                    else:
                        nc.vector.tensor_copy(o_sb[:, nh * NB : (nh + 1) * NB], fo[:, :])
                nc.sync.dma_start(out=out_d[qt * 128 : (qt + 1) * 128, :], in_=o_sb[:, :])

    nc.compile()
    return nc


_NC_CACHE = {}


def _get_nc():
    if "nc" not in _NC_CACHE:
        _NC_CACHE["nc"] = build_attention_nc()
    return _NC_CACHE["nc"]


def kernel(x, mask, Wq, Wk, Wv, Wo, bo):
    from concourse.bass_utils import run_bass_kernel_spmd

    x = np.asarray(x, dtype=np.float32)
    mask = np.asarray(mask)
    Wq = np.asarray(Wq, dtype=np.float32)
    Wk = np.asarray(Wk, dtype=np.float32)
    Wv = np.asarray(Wv, dtype=np.float32)
    Wo = np.asarray(Wo, dtype=np.float32)
    bo = np.asarray(bo, dtype=np.float32)

    B, S, D = x.shape
    G = 4  # head-groups per batch
    INNER = 256  # head-group inner width

    maskT_by_b = {}
    in_maps = []
    for c in range(8):
        b, g = c // G, c % G
        if b not in maskT_by_b:
            maskT_by_b[b] = np.ascontiguousarray(mask[b].T).astype(ml_dtypes.bfloat16)
        cols = slice(g * INNER, (g + 1) * INNER)
        in_maps.append(
            {
                "xT": np.ascontiguousarray(x[b].T),
                "maskT": maskT_by_b[b],
                "Wq": np.ascontiguousarray(Wq[:, cols]),
                "Wk": np.ascontiguousarray(Wk[:, cols]),
                "Wv": np.ascontiguousarray(Wv[:, cols]),
                "Wo": np.ascontiguousarray(Wo[cols, :]),
            }
        )

    res = run_bass_kernel_spmd(_get_nc(), in_maps, core_ids=list(range(8)))
    outs = [r["out"] for r in res.results]
    full = np.empty((B, S, D), dtype=np.float32)
    for b in range(B):
        acc = outs[b * G].astype(np.float32, copy=True)
        for g in range(1, G):
            acc += outs[b * G + g]
        full[b] = acc + bo[None, :]
    return full
